# revision 1
# baseline (speedup 1.0000x reference)
"""Debiased EMA (nn_DebiasedEMA) Trainium2 Bass kernel.

x: [B=32, T=4096, C=512] f32.
    y_t = a*y_{t-1} + (1-a)*x_t  (y_0 = x_0), a = f32(0.9)
    out_t = y_t / max(1 - a^(t+1), 1e-6)

Math: a^256 ~ 2e-12, so to fp32 precision y_t depends only on the last 256
inputs.  Each 128-row time block of the output is therefore an exact (to f32)
pair of matmuls in the *natural* [T, C] layout:

    out_block_i = P.T @ x_block_{i-1} + C.T @ x_block_i

with C[k, m] = (1-a) * a^(m-k) for m >= k (triangular, current block) and
P[k, m] = (1-a) * a^(128+m-k) (previous-block tail).  The debias divisor is
exactly 1.0 in fp32 for t >= 165, so it is folded into special weight
matrices for blocks 0 and 1 only (block 0 also carries the y_0 = x_0
initial-condition column).  Blocks 2..31 all share one (P, C) pair.

Sharding: batch-parallel, 4 sequences per NeuronCore, no communication.
"""

import sys

for _p in ("/opt/trn_rl_repo", "/opt/pypackages"):
    if _p not in sys.path:
        sys.path.insert(0, _p)

import numpy as np

import concourse.bacc as bacc
import concourse.mybir as mybir
from concourse import bass_utils
from concourse.tile import TileContext

B, T, C = 32, 4096, 512
NCORES = 8
BPC = B // NCORES        # sequences per core
L = 128                  # time-block length == partition count
ALPHA = 0.9
DENOM_MIN = 1e-6

F32 = mybir.dt.float32


def _build_weights() -> np.ndarray:
    """Five 128x128 lhsT weight matrices, packed [128, 640] f32.

    lhsT layout: [k (input time, partitions), m (output time, free)];
    matmul computes out[m, n] = sum_k lhsT[k, m] * x[k, n].
    Built in float64 from the f32-rounded alpha, rounded once to f32.
    """
    a = float(np.float32(ALPHA))     # f32 value of clip(0.9) as f64
    omb = 1.0 - a                    # exact (Sterbenz), matches f32 1-a
    k = np.arange(L, dtype=np.float64)[:, None]   # input index
    m = np.arange(L, dtype=np.float64)[None, :]   # output index
    tri = (m - k) >= 0
    # debias divisors d[t] = max(1 - a^(t+1), DENOM_MIN) for t = 0..255
    t = np.arange(2 * L, dtype=np.float64)
    d = np.maximum(1.0 - a ** (t + 1.0), DENOM_MIN)

    dec = np.where(tri, a ** np.where(tri, m - k, 0.0), 0.0)     # a^(m-k)
    x0col = (k == 0)
    # block 0: y_m = a^m x_0 + (1-a) sum_{s=1..m} a^(m-s) x_s, row-scaled 1/d[m]
    A0 = np.where(tri, np.where(x0col, a**m, omb * dec), 0.0) / d[:L][None, :]
    # block 1: prev-block (= block 0, incl. x_0 column) + current, /d[128+m]
    P1 = np.where(x0col, a ** (128.0 + m), omb * a ** (128.0 + m - k)) \
        / d[L:][None, :]
    C1 = omb * dec / d[L:][None, :]
    # blocks >= 2 (debias == 1.0 exactly in f32)
    P = omb * a ** (128.0 + m - k)
    Cm = omb * dec
    w = np.concatenate([A0, P1, C1, P, Cm], axis=1)
    return np.ascontiguousarray(w.astype(np.float32))


def build_program(bpc: int = BPC, t_len: int = T, chunk: int = 8):
    """One core's program: EMA over `bpc` independent [t_len, C] sequences."""
    nblk = t_len // L
    nchunk = nblk // chunk
    assert nblk * L == t_len and nchunk * chunk == nblk

    nc = bacc.Bacc("TRN2", target_bir_lowering=False, debug=False)
    x = nc.dram_tensor("x", [bpc * t_len, C], F32, kind="ExternalInput").ap()
    w = nc.dram_tensor("w", [L, 5 * L], F32, kind="ExternalInput").ap()
    y = nc.dram_tensor("y", [bpc * t_len, C], F32, kind="ExternalOutput").ap()

    with TileContext(nc) as tc:
        with (
            tc.tile_pool(name="wpool", bufs=1) as wpool,
            tc.tile_pool(name="xpool", bufs=3) as xpool,
            tc.tile_pool(name="ypool", bufs=3) as ypool,
            tc.tile_pool(name="psum", bufs=8, space="PSUM") as ppool,
        ):
            wt = wpool.tile([L, 5 * L], F32)
            nc.sync.dma_start(out=wt[:, :], in_=w[:, :])
            A0w = wt[:, 0 * L:1 * L]
            P1w = wt[:, 1 * L:2 * L]
            C1w = wt[:, 2 * L:3 * L]
            Pw = wt[:, 3 * L:4 * L]
            Cw = wt[:, 4 * L:5 * L]

            eng_i = 0
            for b in range(bpc):
                prev_chunk = None
                for ch in range(nchunk):
                    r0 = b * t_len + ch * chunk * L
                    xt = xpool.tile([L, chunk * C], F32)
                    nc.sync.dma_start(
                        out=xt[:, :].rearrange("p (n c) -> p n c", c=C),
                        in_=x[r0:r0 + chunk * L, :].rearrange(
                            "(n p) c -> p n c", p=L),
                    )
                    yt = ypool.tile([L, chunk * C], F32)
                    for j in range(chunk):
                        i = ch * chunk + j   # block index within the sequence
                        cur = xt[:, j * C:(j + 1) * C]
                        ps = ppool.tile([L, C], F32)
                        if i == 0:
                            nc.tensor.matmul(ps[:, :], A0w, cur,
                                             start=True, stop=True)
                        else:
                            prev = (xt[:, (j - 1) * C:j * C] if j > 0
                                    else prev_chunk[:, (chunk - 1) * C:])
                            pw, cw = (P1w, C1w) if i == 1 else (Pw, Cw)
                            nc.tensor.matmul(ps[:, :], pw, prev,
                                             start=True, stop=False)
                            nc.tensor.matmul(ps[:, :], cw, cur,
                                             start=False, stop=True)
                        # PSUM -> SBUF copy, alternating engines for balance
                        dst = yt[:, j * C:(j + 1) * C]
                        if eng_i % 2 == 0:
                            nc.vector.tensor_copy(out=dst, in_=ps[:, :])
                        else:
                            nc.scalar.copy(dst, ps[:, :])
                        eng_i += 1
                    nc.sync.dma_start(
                        out=y[r0:r0 + chunk * L, :].rearrange(
                            "(n p) c -> p n c", p=L),
                        in_=yt[:, :].rearrange("p (n c) -> p n c", c=C),
                    )
                    prev_chunk = xt
    nc.compile()
    return nc


_CACHE: dict = {}


def _get_program():
    if "nc" not in _CACHE:
        _CACHE["nc"] = build_program()
        _CACHE["w"] = _build_weights()
    return _CACHE["nc"], _CACHE["w"]


def _run(x: np.ndarray, trace: bool = False):
    nc, w = _get_program()
    in_maps = [
        {
            "x": np.ascontiguousarray(
                x[k * BPC:(k + 1) * BPC].reshape(BPC * T, C)),
            "w": w,
        }
        for k in range(NCORES)
    ]
    res = bass_utils.run_bass_kernel_spmd(
        nc, in_maps, core_ids=list(range(NCORES)), trace=trace)
    y = np.concatenate(
        [r["y"].reshape(BPC, T, C) for r in res.results], axis=0)
    return y, res


def kernel(x) -> np.ndarray:
    x = np.asarray(x, dtype=np.float32)
    assert x.shape == (B, T, C), x.shape
    y, _ = _run(x, trace=False)
    return y
